# revision 8
# baseline (speedup 1.0000x reference)
"""Trainium2 Bass kernel for nn_BEMBFlex (within-category log-softmax utility model).

Strategy: shard ITEMS BY CATEGORY across the 8 cores (categories never span
shards), session-major layout [128 sessions x item-cols] per chunk.

Device computes, per (chunk, block):
  1. PE:  psum = thzet_j^T @ W_blk (utility). For "PE-lambda" blocks
     (~60% of cols) a 2-row accumulating matmul adds [lam_hi; lam_lo]
     (bf16 hi/lo, ~6e-5-exact) so psum = util + lambda there.
  2. ACT: ex = Exp(psum) -> bf16 SBUF straight from PSUM; PSUM frees here.
     (The ONLY ACT function: one table set, no reload thrash.)
  3. DVE: for non-PE-lambda blocks, ew = ex * wexp (wexp = e^lambda
     DMA-broadcast row, bf16 2x mode). GPSIMD is left idle on purpose:
     it shares the physical SBUF port with the DVE, and measured offload
     inflated concurrent DVE ops ~6x.
  4. DVE: two bf16 halving adds then the 1x reduce -> s[g] = within-slot
     weighted sums.
  5. DMA out ex (bf16) and s (f32).

Host finishes: out = log(ex) [+ lambda where not folded] - log(s)[category]
in f32 plus the de-permutation gather. This is exact math (log/gather are
elementwise/indexing); everything O(B*I) heavy (matmul, exp, segment
reduce) runs on device.

Range safety (no max-subtraction needed): |util+lam| <~ 85 so ex <= 8e36
< bf16 max 3.4e38; weighted sums <= 6e36 stay f32-normal; weakest
category maxima ~e^-55 stay normal. log on host is f32-exact.
"""

import sys

for _p in ("/opt/trn_rl_repo",):
    if _p not in sys.path:
        sys.path.insert(0, _p)

import ml_dtypes
import numpy as np

import concourse.bass as bass
import concourse.tile as tile
from concourse import bacc, bass_utils, mybir

NUM_USERS = 100000
NUM_ITEMS = 25000
NUM_CATS = 500
LATENT = 64
BATCH = 1024
NCORES = 8
P = 128                    # partitions / sessions per matmul chunk
NCHUNKS = BATCH // P       # session chunks per core
BLOCK_COLS = 1024          # max padded cols per processing block (2 PSUM banks)
FIRST_COLS = 512           # small first block so the pipeline starts early
PE_LAM_FRAC = 0.50         # target fraction of cols with lambda folded on PE
MM_COLS = 512              # matmul piece width (1 PSUM bank)

F32 = mybir.dt.float32
BF16 = mybir.dt.bfloat16

_nc_cache = {}


# ----------------------------------------------------------------------------
# Host-side layout
# ----------------------------------------------------------------------------

def _layout(cat_sizes):
    """Slot/block layout shared by all 8 shards.

    Categories sorted by size desc; slot i holds category ranks
    [8i, 8i+8) (one per shard). Slot width L_i = first (largest) size in
    the group rounded up to a multiple of 8 (so two bf16 halving passes
    stay 4B-aligned). Blocks greedily group consecutive slots under a
    uniform L with g*L <= cap.
    """
    order = np.argsort(-cat_sizes, kind="stable")
    order = order[cat_sizes[order] > 0]
    ncats = len(order)
    nslots = -(-ncats // NCORES)
    slot_L = np.empty(nslots, np.int64)
    for i in range(nslots):
        mx = int(cat_sizes[order[i * NCORES]])
        slot_L[i] = max(8, ((mx + 7) // 8) * 8)
    blocks = []  # (col0, g, L, slot0)
    col = 0
    i = 0
    while i < nslots:
        Lb = int(slot_L[i])
        cap = FIRST_COLS if not blocks else BLOCK_COLS
        g = 1
        while i + g < nslots and (g + 1) * Lb <= cap:
            g += 1
        blocks.append((col, g, Lb, i))
        col += g * Lb
        i += g
    ipad = col
    slot_col = np.empty(nslots, np.int64)
    for (c0, g, Lb, s0) in blocks:
        for q in range(g):
            slot_col[s0 + q] = c0 + q * Lb
    return order, blocks, ipad, slot_col


def _pick_pe_lam_blocks(blocks):
    """Best-fit subset of blocks totalling ~PE_LAM_FRAC of columns whose
    lambda folds on the PE (the rest use the DVE multiply)."""
    total = sum(g * L for (_c, g, L, _s) in blocks)
    target = PE_LAM_FRAC * total
    best, best_err = frozenset(), abs(target)
    n = len(blocks)
    for mask in range(1 << n):
        acc = sum(blocks[i][1] * blocks[i][2] for i in range(n) if mask >> i & 1)
        err = abs(acc - target)
        if err < best_err:
            best_err = err
            best = frozenset(i for i in range(n) if mask >> i & 1)
    return best


def _prep(inputs):
    cat = np.asarray(inputs["category_idx"]).astype(np.int64).ravel()
    cat_sizes = np.bincount(cat, minlength=NUM_CATS)
    order, blocks, ipad, slot_col = _layout(cat_sizes)
    nslots = len(slot_col)

    rank = np.full(NUM_CATS, -1, np.int64)
    rank[order] = np.arange(len(order))

    # position of each item within its category (stable order)
    perm = np.argsort(cat, kind="stable")
    starts = np.searchsorted(cat[perm], np.arange(NUM_CATS))
    within_sorted = np.arange(NUM_ITEMS) - starts[cat[perm]]
    item_within = np.empty(NUM_ITEMS, np.int64)
    item_within[perm] = within_sorted

    r = rank[cat]
    item_shard = r % NCORES
    item_slot = r // NCORES
    item_col = slot_col[item_slot] + item_within

    alpha = np.ascontiguousarray(np.asarray(inputs["alpha_item"], np.float32))
    obs = np.ascontiguousarray(np.asarray(inputs["item_obs"], np.float32))
    lam = np.asarray(inputs["lambda_item"], np.float32).ravel()

    W = np.zeros((NCORES, 2 * LATENT, ipad), np.float32)
    LAMF = np.zeros((NCORES, ipad), np.float32)
    WEXP = np.zeros((NCORES, 1, ipad), np.float32)
    for s in range(NCORES):
        m = item_shard == s
        cols = item_col[m]
        W[s, 0:LATENT, cols] = alpha[m]
        W[s, LATENT:, cols] = obs[m]
        LAMF[s, cols] = lam[m]
        WEXP[s, 0, cols] = np.exp(lam[m])
    W = W.astype(ml_dtypes.bfloat16)
    lam_hi = LAMF.astype(ml_dtypes.bfloat16)
    lam_lo = (LAMF - lam_hi.astype(np.float32)).astype(ml_dtypes.bfloat16)
    LAM2 = np.stack([lam_hi, lam_lo], axis=1)  # [NCORES, 2, ipad] bf16
    WEXP = WEXP.astype(ml_dtypes.bfloat16)

    uidx = np.asarray(inputs["user_index"]).astype(np.int64).ravel()
    theta = np.asarray(inputs["theta_user"], np.float32)
    zeta = np.asarray(inputs["zeta_user"], np.float32)
    thzet = np.ascontiguousarray(
        np.concatenate([theta[uidx], zeta[uidx]], axis=1).T
    ).astype(ml_dtypes.bfloat16)
    ones2 = np.ones((2, P), ml_dtypes.bfloat16)
    return {
        "blocks": blocks,
        "ipad": ipad,
        "nslots": nslots,
        "item_shard": item_shard,
        "item_slot": item_slot,
        "item_col": item_col,
        "lam": lam,
        "W": W,
        "LAM2": LAM2,
        "WEXP": WEXP,
        "thzet": thzet,
        "ones2": ones2,
    }


# ----------------------------------------------------------------------------
# Device program
# ----------------------------------------------------------------------------

def _3d(t2d, g, L):
    """[P, g*L] tile -> [P, g, L] AP."""
    ap = t2d[:, :]
    return bass.AP(tensor=ap.tensor, offset=ap.offset,
                   ap=[ap.ap[0], [L, g], [1, L]])


def _half_aps_ap(ap, g, L):
    """[P, g*L] AP -> two [P, g, L/2] read-APs (low and high halves)."""
    h = L // 2
    lo = bass.AP(tensor=ap.tensor, offset=ap.offset,
                 ap=[ap.ap[0], [L, g], [1, h]])
    hi = bass.AP(tensor=ap.tensor, offset=ap.offset + h,
                 ap=[ap.ap[0], [L, g], [1, h]])
    return lo, hi


def _half_aps(t2d, g, L):
    """[P, g*L] tile -> two [P, g, L/2] read-APs (low and high halves)."""
    ap = t2d[:, :]
    h = L // 2
    lo = bass.AP(tensor=ap.tensor, offset=ap.offset,
                 ap=[ap.ap[0], [L, g], [1, h]])
    hi = bass.AP(tensor=ap.tensor, offset=ap.offset + h,
                 ap=[ap.ap[0], [L, g], [1, h]])
    return lo, hi


def _build_nc(blocks, ipad, nslots, pe_lam_blocks):
    nc = bacc.Bacc(
        "TRN2",
        debug=False,
        enable_asserts=False,
        target_bir_lowering=False,
        num_devices=NCORES,
    )
    w_d = nc.dram_tensor("W", [2 * LATENT, ipad], BF16, kind="ExternalInput").ap()
    lam2_d = nc.dram_tensor("LAM2", [2, ipad], BF16, kind="ExternalInput").ap()
    wexp_d = nc.dram_tensor("WEXP", [1, ipad], BF16, kind="ExternalInput").ap()
    thzet_d = nc.dram_tensor("THZET", [2 * LATENT, BATCH], BF16, kind="ExternalInput").ap()
    ones2_d = nc.dram_tensor("ONES2", [2, P], BF16, kind="ExternalInput").ap()
    ex_d = nc.dram_tensor("EX", [BATCH, ipad], BF16, kind="ExternalOutput").ap()
    s_d = nc.dram_tensor("S", [BATCH, nslots], F32, kind="ExternalOutput").ap()

    with tile.TileContext(nc) as tc:
        with (
            tc.tile_pool(name="singles", bufs=1) as singles,
            tc.tile_pool(name="psum_u", bufs=3, space="PSUM") as psum_u,
            tc.tile_pool(name="exbuf", bufs=2) as exbuf,
            tc.tile_pool(name="ewbuf", bufs=4) as ewbuf,
            tc.tile_pool(name="h1buf", bufs=4) as h1buf,
            tc.tile_pool(name="h2buf", bufs=4) as h2buf,
            tc.tile_pool(name="stats", bufs=8) as stats,
        ):
            # per-block / per-chunk input tiles: separate tiles keep the
            # dependency tracking fine-grained so the first matmul starts
            # as soon as its own W block + session slice have landed
            w_bs = {}
            lam2_bs = {}
            wexp_bs = {}
            thze_t = []
            (c0f, gf, Lf, _sf) = blocks[0]
            w_bs[0] = singles.tile([2 * LATENT, gf * Lf], BF16, name="w_b0")
            nc.sync.dma_start(out=w_bs[0][:, :], in_=w_d[:, 0:gf * Lf])
            t0 = singles.tile([2 * LATENT, P], BF16, name="thzet_0")
            nc.sync.dma_start(out=t0[:, :], in_=thzet_d[:, 0:P])
            thze_t.append(t0[:, :])
            ones2_sb = singles.tile([2, P], BF16, name="ones2_sb")
            nc.sync.dma_start(out=ones2_sb[:, :], in_=ones2_d[:, :])
            for j in range(1, NCHUNKS):
                tj = singles.tile([2 * LATENT, P], BF16, name=f"thzet_{j}")
                nc.sync.dma_start(
                    out=tj[:, :], in_=thzet_d[:, j * P:(j + 1) * P]
                )
                thze_t.append(tj[:, :])
            for bi, (col0, g, L, _s0) in enumerate(blocks):
                cols = g * L
                if bi > 0:
                    w_bs[bi] = singles.tile(
                        [2 * LATENT, cols], BF16, name=f"w_b{bi}"
                    )
                    nc.sync.dma_start(
                        out=w_bs[bi][:, :], in_=w_d[:, col0:col0 + cols]
                    )
                if bi in pe_lam_blocks:
                    lam2_bs[bi] = singles.tile([2, cols], BF16, name=f"lam2_b{bi}")
                    nc.sync.dma_start(
                        out=lam2_bs[bi][:, :], in_=lam2_d[:, col0:col0 + cols]
                    )
                else:
                    # e^lambda broadcast to all 128 partitions
                    wexp_bs[bi] = singles.tile([P, cols], BF16, name=f"wexp_b{bi}")
                    nc.sync.dma_start(
                        out=wexp_bs[bi][:, :],
                        in_=bass.AP(
                            tensor=wexp_d.tensor, offset=col0,
                            ap=[[0, P], [1, cols]],
                        ),
                    )

            for j in range(NCHUNKS):
                ex = exbuf.tile([P, ipad], BF16, name="ex", tag="ex")
                s_all = stats.tile([P, nslots], F32, name="s_all", tag="s_all")
                for bi, (col0, g, L, s0) in enumerate(blocks):
                    cols = g * L
                    pe_lam = bi in pe_lam_blocks
                    up = psum_u.tile([P, cols], F32, name="up", tag="up")
                    for c0 in range(0, cols, MM_COLS):
                        cn = min(MM_COLS, cols - c0)
                        nc.tensor.matmul(
                            up[:, c0:c0 + cn],
                            lhsT=thze_t[j],
                            rhs=w_bs[bi][:, c0:c0 + cn],
                            start=True,
                            stop=not pe_lam,
                        )
                    if pe_lam:
                        for c0 in range(0, cols, MM_COLS):
                            cn = min(MM_COLS, cols - c0)
                            nc.tensor.matmul(
                                up[:, c0:c0 + cn],
                                lhsT=ones2_sb[:, :],
                                rhs=lam2_bs[bi][:, c0:c0 + cn],
                                start=False,
                                stop=True,
                            )
                    # ex = exp(psum) -> bf16; PSUM frees after this
                    exb = ex[:, col0:col0 + cols]
                    nc.scalar.activation(
                        out=exb, in_=up[:, :],
                        func=mybir.ActivationFunctionType.Exp,
                    )
                    if pe_lam:
                        red_in = exb
                    else:
                        # weight by e^lambda before the within-slot reduce
                        ew = ewbuf.tile([P, cols], BF16, name="ew", tag="ew")
                        nc.vector.tensor_tensor(
                            out=ew[:, :], in0=exb,
                            in1=wexp_bs[bi][:, :],
                            op=mybir.AluOpType.mult,
                        )
                        red_in = ew[:, :]
                    # two bf16 halving adds (2x DVE) then the 1x reduce
                    h1 = h1buf.tile([P, cols // 2], BF16, name="h1", tag="h1")
                    lo, hi = _half_aps_ap(red_in, g, L)
                    nc.vector.tensor_add(out=_3d(h1, g, L // 2), in0=lo, in1=hi)
                    h2 = h2buf.tile([P, cols // 4], BF16, name="h2", tag="h2")
                    lo2, hi2 = _half_aps(h1, g, L // 2)
                    nc.vector.tensor_add(out=_3d(h2, g, L // 4), in0=lo2, in1=hi2)
                    nc.vector.reduce_sum(
                        out=s_all[:, s0:s0 + g],
                        in_=_3d(h2, g, L // 4),
                        axis=mybir.AxisListType.X,
                    )
                nc.sync.dma_start(
                    out=ex_d[j * P:(j + 1) * P, :],
                    in_=ex[:, :],
                )
                nc.sync.dma_start(
                    out=s_d[j * P:(j + 1) * P, :],
                    in_=s_all[:, :],
                )
    nc.compile()
    return nc


# ----------------------------------------------------------------------------
# Entry points
# ----------------------------------------------------------------------------

def run(inputs, trace=False):
    prep = _prep(inputs)
    blocks = prep["blocks"]
    pe_lam_blocks = _pick_pe_lam_blocks(blocks)
    key = (prep["ipad"], tuple(blocks), tuple(sorted(pe_lam_blocks)))
    nc = _nc_cache.get(key)
    if nc is None:
        nc = _build_nc(blocks, prep["ipad"], prep["nslots"], pe_lam_blocks)
        _nc_cache[key] = nc
    in_maps = [
        {
            "W": prep["W"][c],
            "LAM2": prep["LAM2"][c],
            "WEXP": prep["WEXP"][c],
            "THZET": prep["thzet"],
            "ONES2": prep["ones2"],
        }
        for c in range(NCORES)
    ]
    res = bass_utils.run_bass_kernel_spmd(
        nc, in_maps, core_ids=list(range(NCORES)), trace=trace
    )
    # host finish: out = log(ex) [+ lam where not folded] - log(s)[cat]
    exs = np.stack([np.asarray(res.results[c]["EX"]) for c in range(NCORES)])
    ss = np.stack([np.asarray(res.results[c]["S"]) for c in range(NCORES)])
    ish, icol, islot = prep["item_shard"], prep["item_col"], prep["item_slot"]
    ex_cols = exs[ish, :, icol].astype(np.float32)      # [I, B]
    out = np.log(ex_cols)
    col_in_pe = np.zeros(prep["ipad"], bool)
    for bi in pe_lam_blocks:
        c0, g, L, _s0 = blocks[bi]
        col_in_pe[c0:c0 + g * L] = True
    need_lam = ~col_in_pe[icol]
    out[need_lam] += prep["lam"][need_lam][:, None]
    out -= np.log(ss[ish, :, islot])
    return np.ascontiguousarray(out.T), res


def kernel(**inputs) -> np.ndarray:
    out, _ = run(inputs, trace=False)
    return out


# revision 11
# speedup vs baseline: 1.1294x; 1.1294x over previous
"""Trainium2 Bass kernel for nn_BEMBFlex (within-category log-softmax utility model).

Strategy: shard ITEMS BY CATEGORY across the 8 cores (categories never span
shards), session-major layout [128 sessions x item-cols] per chunk.

Device computes, per (chunk, block):
  1. PE:  psum = thzet_j^T @ W_blk (utility). For "PE-lambda" blocks
     (~60% of cols) a 2-row accumulating matmul adds [lam_hi; lam_lo]
     (bf16 hi/lo, ~6e-5-exact) so psum = util + lambda there.
  2. ACT: ex = Exp(psum) -> bf16 SBUF straight from PSUM; PSUM frees here.
     (The ONLY ACT function: one table set, no reload thrash.)
  3. DVE: for non-PE-lambda blocks, ew = ex * wexp (wexp = e^lambda
     DMA-broadcast row, bf16 2x mode). GPSIMD is left idle on purpose:
     it shares the physical SBUF port with the DVE, and measured offload
     inflated concurrent DVE ops ~6x.
  4. DVE: two bf16 halving adds then the 1x reduce -> s[g] = within-slot
     weighted sums.
  5. DMA out ex (bf16) and s (f32).

Host finishes: out = log(ex) [+ lambda where not folded] - log(s)[category]
in f32 plus the de-permutation gather. This is exact math (log/gather are
elementwise/indexing); everything O(B*I) heavy (matmul, exp, segment
reduce) runs on device.

Range safety (no max-subtraction needed): |util+lam| <~ 85 so ex <= 8e36
< bf16 max 3.4e38; weighted sums <= 6e36 stay f32-normal; weakest
category maxima ~e^-55 stay normal. log on host is f32-exact.
"""

import sys

for _p in ("/opt/trn_rl_repo",):
    if _p not in sys.path:
        sys.path.insert(0, _p)

import ml_dtypes
import numpy as np

import concourse.bass as bass
import concourse.tile as tile
from concourse import bacc, bass_utils, mybir

NUM_USERS = 100000
NUM_ITEMS = 25000
NUM_CATS = 500
LATENT = 64
BATCH = 1024
NCORES = 8
P = 128                    # partitions / sessions per matmul chunk
NCHUNKS = BATCH // P       # session chunks per core
BLOCK_COLS = 1024          # max padded cols per processing block (2 PSUM banks)
FIRST_COLS = 512           # small first block so the pipeline starts early
PE_LAM_FRAC = 0.45         # target fraction of cols with lambda folded on PE
MM_COLS = 512              # matmul piece width (1 PSUM bank)

F32 = mybir.dt.float32
BF16 = mybir.dt.bfloat16

_nc_cache = {}


# ----------------------------------------------------------------------------
# Host-side layout
# ----------------------------------------------------------------------------

def _layout(cat_sizes):
    """Slot/block layout shared by all 8 shards.

    Categories sorted by size desc; slot i holds category ranks
    [8i, 8i+8) (one per shard). Slot width L_i = first (largest) size in
    the group rounded up to a multiple of 8 (so two bf16 halving passes
    stay 4B-aligned). Blocks greedily group consecutive slots under a
    uniform L with g*L <= cap.
    """
    order = np.argsort(-cat_sizes, kind="stable")
    order = order[cat_sizes[order] > 0]
    ncats = len(order)
    nslots = -(-ncats // NCORES)
    slot_L = np.empty(nslots, np.int64)
    for i in range(nslots):
        mx = int(cat_sizes[order[i * NCORES]])
        slot_L[i] = max(8, ((mx + 7) // 8) * 8)
    blocks = []  # (col0, g, L, slot0)
    col = 0
    i = 0
    while i < nslots:
        Lb = int(slot_L[i])
        cap = FIRST_COLS if not blocks else BLOCK_COLS
        g = 1
        while i + g < nslots and (g + 1) * Lb <= cap:
            g += 1
        blocks.append((col, g, Lb, i))
        col += g * Lb
        i += g
    ipad = col
    slot_col = np.empty(nslots, np.int64)
    for (c0, g, Lb, s0) in blocks:
        for q in range(g):
            slot_col[s0 + q] = c0 + q * Lb
    return order, blocks, ipad, slot_col


def _pick_pe_lam_blocks(blocks):
    """Best-fit subset of blocks totalling ~PE_LAM_FRAC of columns whose
    lambda folds on the PE (the rest use the DVE multiply)."""
    total = sum(g * L for (_c, g, L, _s) in blocks)
    target = PE_LAM_FRAC * total
    best, best_err = frozenset(), abs(target)
    n = len(blocks)
    for mask in range(1 << n):
        acc = sum(blocks[i][1] * blocks[i][2] for i in range(n) if mask >> i & 1)
        err = abs(acc - target)
        if err < best_err:
            best_err = err
            best = frozenset(i for i in range(n) if mask >> i & 1)
    return best


def _prep(inputs):
    cat = np.asarray(inputs["category_idx"]).astype(np.int64).ravel()
    cat_sizes = np.bincount(cat, minlength=NUM_CATS)
    order, blocks, ipad, slot_col = _layout(cat_sizes)
    nslots = len(slot_col)

    rank = np.full(NUM_CATS, -1, np.int64)
    rank[order] = np.arange(len(order))

    # position of each item within its category (stable order)
    perm = np.argsort(cat, kind="stable")
    starts = np.searchsorted(cat[perm], np.arange(NUM_CATS))
    within_sorted = np.arange(NUM_ITEMS) - starts[cat[perm]]
    item_within = np.empty(NUM_ITEMS, np.int64)
    item_within[perm] = within_sorted

    r = rank[cat]
    item_shard = r % NCORES
    item_slot = r // NCORES
    item_col = slot_col[item_slot] + item_within

    alpha = np.ascontiguousarray(np.asarray(inputs["alpha_item"], np.float32))
    obs = np.ascontiguousarray(np.asarray(inputs["item_obs"], np.float32))
    lam = np.asarray(inputs["lambda_item"], np.float32).ravel()

    W = np.zeros((NCORES, 2 * LATENT, ipad), np.float32)
    LAMF = np.zeros((NCORES, ipad), np.float32)
    WEXP = np.zeros((NCORES, 1, ipad), np.float32)
    for s in range(NCORES):
        m = item_shard == s
        cols = item_col[m]
        W[s, 0:LATENT, cols] = alpha[m]
        W[s, LATENT:, cols] = obs[m]
        LAMF[s, cols] = lam[m]
        WEXP[s, 0, cols] = np.exp(lam[m])
    W = W.astype(ml_dtypes.bfloat16)
    lam_hi = LAMF.astype(ml_dtypes.bfloat16)
    lam_lo = (LAMF - lam_hi.astype(np.float32)).astype(ml_dtypes.bfloat16)
    LAM2 = np.stack([lam_hi, lam_lo], axis=1)  # [NCORES, 2, ipad] bf16
    WEXP = WEXP.astype(ml_dtypes.bfloat16)

    uidx = np.asarray(inputs["user_index"]).astype(np.int64).ravel()
    theta = np.asarray(inputs["theta_user"], np.float32)
    zeta = np.asarray(inputs["zeta_user"], np.float32)
    thzet = np.ascontiguousarray(
        np.concatenate([theta[uidx], zeta[uidx]], axis=1).T
    ).astype(ml_dtypes.bfloat16)
    ones2 = np.ones((2, P), ml_dtypes.bfloat16)
    return {
        "blocks": blocks,
        "ipad": ipad,
        "nslots": nslots,
        "item_shard": item_shard,
        "item_slot": item_slot,
        "item_col": item_col,
        "lam": lam,
        "W": W,
        "LAM2": LAM2,
        "WEXP": WEXP,
        "thzet": thzet,
        "ones2": ones2,
    }


# ----------------------------------------------------------------------------
# Device program
# ----------------------------------------------------------------------------

def _3d(t2d, g, L):
    """[P, g*L] tile -> [P, g, L] AP."""
    ap = t2d[:, :]
    return bass.AP(tensor=ap.tensor, offset=ap.offset,
                   ap=[ap.ap[0], [L, g], [1, L]])


def _half_aps_ap(ap, g, L):
    """[P, g*L] AP -> two [P, g, L/2] read-APs (low and high halves)."""
    h = L // 2
    lo = bass.AP(tensor=ap.tensor, offset=ap.offset,
                 ap=[ap.ap[0], [L, g], [1, h]])
    hi = bass.AP(tensor=ap.tensor, offset=ap.offset + h,
                 ap=[ap.ap[0], [L, g], [1, h]])
    return lo, hi


def _half_aps(t2d, g, L):
    """[P, g*L] tile -> two [P, g, L/2] read-APs (low and high halves)."""
    ap = t2d[:, :]
    h = L // 2
    lo = bass.AP(tensor=ap.tensor, offset=ap.offset,
                 ap=[ap.ap[0], [L, g], [1, h]])
    hi = bass.AP(tensor=ap.tensor, offset=ap.offset + h,
                 ap=[ap.ap[0], [L, g], [1, h]])
    return lo, hi


def _build_nc(blocks, ipad, nslots, pe_lam_blocks):
    nc = bacc.Bacc(
        "TRN2",
        debug=False,
        enable_asserts=False,
        target_bir_lowering=False,
        num_devices=NCORES,
    )
    w_d = nc.dram_tensor("W", [2 * LATENT, ipad], BF16, kind="ExternalInput").ap()
    lam2_d = nc.dram_tensor("LAM2", [2, ipad], BF16, kind="ExternalInput").ap()
    wexp_d = nc.dram_tensor("WEXP", [1, ipad], BF16, kind="ExternalInput").ap()
    thzet_d = nc.dram_tensor("THZET", [2 * LATENT, BATCH], BF16, kind="ExternalInput").ap()
    ones2_d = nc.dram_tensor("ONES2", [2, P], BF16, kind="ExternalInput").ap()
    ex_d = nc.dram_tensor("EX", [BATCH, ipad], BF16, kind="ExternalOutput").ap()
    s_d = nc.dram_tensor("S", [BATCH, nslots], F32, kind="ExternalOutput").ap()

    with tile.TileContext(nc) as tc:
        with (
            tc.tile_pool(name="singles", bufs=1) as singles,
            tc.tile_pool(name="psum_u", bufs=3, space="PSUM") as psum_u,
            tc.tile_pool(name="psum_s", bufs=2, space="PSUM") as psum_s,
            tc.tile_pool(name="exbuf", bufs=2) as exbuf,
            tc.tile_pool(name="ewbuf", bufs=4) as ewbuf,
            tc.tile_pool(name="h1buf", bufs=4) as h1buf,
            tc.tile_pool(name="h2buf", bufs=4) as h2buf,
            tc.tile_pool(name="stats", bufs=8) as stats,
        ):
            # per-block / per-chunk input tiles: separate tiles keep the
            # dependency tracking fine-grained so the first matmul starts
            # as soon as its own W block + session slice have landed
            w_bs = {}
            lam2_bs = {}
            wexp_bs = {}
            thze_t = []
            (c0f, gf, Lf, _sf) = blocks[0]
            w_bs[0] = singles.tile([2 * LATENT, gf * Lf], BF16, name="w_b0")
            nc.sync.dma_start(out=w_bs[0][:, :], in_=w_d[:, 0:gf * Lf])
            t0 = singles.tile([2 * LATENT, P], BF16, name="thzet_0")
            nc.sync.dma_start(out=t0[:, :], in_=thzet_d[:, 0:P])
            thze_t.append(t0[:, :])
            ones2_sb = singles.tile([2, P], BF16, name="ones2_sb")
            nc.sync.dma_start(out=ones2_sb[:, :], in_=ones2_d[:, :])
            for j in range(1, NCHUNKS):
                tj = singles.tile([2 * LATENT, P], BF16, name=f"thzet_{j}")
                nc.sync.dma_start(
                    out=tj[:, :], in_=thzet_d[:, j * P:(j + 1) * P]
                )
                thze_t.append(tj[:, :])
            for bi, (col0, g, L, _s0) in enumerate(blocks):
                cols = g * L
                if bi > 0:
                    w_bs[bi] = singles.tile(
                        [2 * LATENT, cols], BF16, name=f"w_b{bi}"
                    )
                    nc.sync.dma_start(
                        out=w_bs[bi][:, :], in_=w_d[:, col0:col0 + cols]
                    )
                if bi in pe_lam_blocks:
                    lam2_bs[bi] = singles.tile([2, cols], BF16, name=f"lam2_b{bi}")
                    nc.sync.dma_start(
                        out=lam2_bs[bi][:, :], in_=lam2_d[:, col0:col0 + cols]
                    )
                else:
                    # e^lambda broadcast to all 128 partitions
                    wexp_bs[bi] = singles.tile([P, cols], BF16, name=f"wexp_b{bi}")
                    nc.sync.dma_start(
                        out=wexp_bs[bi][:, :],
                        in_=bass.AP(
                            tensor=wexp_d.tensor, offset=col0,
                            ap=[[0, P], [1, cols]],
                        ),
                    )

            for j in range(NCHUNKS):
                ex = exbuf.tile([P, ipad], BF16, name="ex", tag="ex")
                s_all = stats.tile([P, nslots], F32, name="s_all", tag="s_all")
                for bi, (col0, g, L, s0) in enumerate(blocks):
                    cols = g * L
                    pe_lam = bi in pe_lam_blocks
                    pool = psum_s if cols <= 512 else psum_u
                    up = pool.tile([P, cols], F32, name="up", tag="up")
                    for c0 in range(0, cols, MM_COLS):
                        cn = min(MM_COLS, cols - c0)
                        nc.tensor.matmul(
                            up[:, c0:c0 + cn],
                            lhsT=thze_t[j],
                            rhs=w_bs[bi][:, c0:c0 + cn],
                            start=True,
                            stop=not pe_lam,
                        )
                    if pe_lam:
                        for c0 in range(0, cols, MM_COLS):
                            cn = min(MM_COLS, cols - c0)
                            nc.tensor.matmul(
                                up[:, c0:c0 + cn],
                                lhsT=ones2_sb[:, :],
                                rhs=lam2_bs[bi][:, c0:c0 + cn],
                                start=False,
                                stop=True,
                            )
                    # ex = exp(psum) -> bf16; PSUM frees after this
                    exb = ex[:, col0:col0 + cols]
                    nc.scalar.activation(
                        out=exb, in_=up[:, :],
                        func=mybir.ActivationFunctionType.Exp,
                    )
                    if pe_lam:
                        red_in = exb
                    else:
                        # weight by e^lambda before the within-slot reduce
                        ew = ewbuf.tile([P, cols], BF16, name="ew", tag="ew")
                        nc.vector.tensor_tensor(
                            out=ew[:, :], in0=exb,
                            in1=wexp_bs[bi][:, :],
                            op=mybir.AluOpType.mult,
                        )
                        red_in = ew[:, :]
                    # two bf16 halving adds (2x DVE) then the 1x reduce
                    h1 = h1buf.tile([P, cols // 2], BF16, name="h1", tag="h1")
                    lo, hi = _half_aps_ap(red_in, g, L)
                    nc.vector.tensor_add(out=_3d(h1, g, L // 2), in0=lo, in1=hi)
                    h2 = h2buf.tile([P, cols // 4], BF16, name="h2", tag="h2")
                    lo2, hi2 = _half_aps(h1, g, L // 2)
                    nc.vector.tensor_add(out=_3d(h2, g, L // 4), in0=lo2, in1=hi2)
                    nc.vector.reduce_sum(
                        out=s_all[:, s0:s0 + g],
                        in_=_3d(h2, g, L // 4),
                        axis=mybir.AxisListType.X,
                    )
                nc.sync.dma_start(
                    out=ex_d[j * P:(j + 1) * P, :],
                    in_=ex[:, :],
                )
                nc.sync.dma_start(
                    out=s_d[j * P:(j + 1) * P, :],
                    in_=s_all[:, :],
                )
    nc.compile()
    return nc


# ----------------------------------------------------------------------------
# Entry points
# ----------------------------------------------------------------------------

def run(inputs, trace=False):
    prep = _prep(inputs)
    blocks = prep["blocks"]
    pe_lam_blocks = _pick_pe_lam_blocks(blocks)
    key = (prep["ipad"], tuple(blocks), tuple(sorted(pe_lam_blocks)))
    nc = _nc_cache.get(key)
    if nc is None:
        nc = _build_nc(blocks, prep["ipad"], prep["nslots"], pe_lam_blocks)
        _nc_cache[key] = nc
    in_maps = [
        {
            "W": prep["W"][c],
            "LAM2": prep["LAM2"][c],
            "WEXP": prep["WEXP"][c],
            "THZET": prep["thzet"],
            "ONES2": prep["ones2"],
        }
        for c in range(NCORES)
    ]
    res = bass_utils.run_bass_kernel_spmd(
        nc, in_maps, core_ids=list(range(NCORES)), trace=trace
    )
    # host finish: out = log(ex) [+ lam where not folded] - log(s)[cat]
    exs = np.stack([np.asarray(res.results[c]["EX"]) for c in range(NCORES)])
    ss = np.stack([np.asarray(res.results[c]["S"]) for c in range(NCORES)])
    ish, icol, islot = prep["item_shard"], prep["item_col"], prep["item_slot"]
    ex_cols = exs[ish, :, icol].astype(np.float32)      # [I, B]
    out = np.log(ex_cols)
    col_in_pe = np.zeros(prep["ipad"], bool)
    for bi in pe_lam_blocks:
        c0, g, L, _s0 = blocks[bi]
        col_in_pe[c0:c0 + g * L] = True
    need_lam = ~col_in_pe[icol]
    out[need_lam] += prep["lam"][need_lam][:, None]
    out -= np.log(ss[ish, :, islot])
    return np.ascontiguousarray(out.T), res


def kernel(**inputs) -> np.ndarray:
    out, _ = run(inputs, trace=False)
    return out


# revision 12
# speedup vs baseline: 1.1297x; 1.0002x over previous
"""Trainium2 Bass kernel for nn_BEMBFlex (within-category log-softmax utility model).

Strategy: shard ITEMS BY CATEGORY across the 8 cores (categories never span
shards), session-major layout [128 sessions x item-cols] per chunk.

Device computes, per (chunk, block):
  1. PE:  psum = thzet_j^T @ W_blk (utility). For "PE-lambda" blocks
     (~60% of cols) a 2-row accumulating matmul adds [lam_hi; lam_lo]
     (bf16 hi/lo, ~6e-5-exact) so psum = util + lambda there.
  2. ACT: ex = Exp(psum) -> bf16 SBUF straight from PSUM; PSUM frees here.
     (The ONLY ACT function: one table set, no reload thrash.)
  3. DVE: for non-PE-lambda blocks, ew = ex * wexp (wexp = e^lambda
     DMA-broadcast row, bf16 2x mode). GPSIMD is left idle on purpose:
     it shares the physical SBUF port with the DVE, and measured offload
     inflated concurrent DVE ops ~6x.
  4. DVE: two bf16 halving adds then the 1x reduce -> s[g] = within-slot
     weighted sums.
  5. DMA out ex (bf16) and s (f32).

Host finishes: out = log(ex) [+ lambda where not folded] - log(s)[category]
in f32 plus the de-permutation gather. This is exact math (log/gather are
elementwise/indexing); everything O(B*I) heavy (matmul, exp, segment
reduce) runs on device.

Range safety (no max-subtraction needed): |util+lam| <~ 85 so ex <= 8e36
< bf16 max 3.4e38; weighted sums <= 6e36 stay f32-normal; weakest
category maxima ~e^-55 stay normal. log on host is f32-exact.
"""

import sys

for _p in ("/opt/trn_rl_repo",):
    if _p not in sys.path:
        sys.path.insert(0, _p)

import ml_dtypes
import numpy as np

import concourse.bass as bass
import concourse.tile as tile
from concourse import bacc, bass_utils, mybir

NUM_USERS = 100000
NUM_ITEMS = 25000
NUM_CATS = 500
LATENT = 64
BATCH = 1024
NCORES = 8
P = 128                    # partitions / sessions per matmul chunk
NCHUNKS = BATCH // P       # session chunks per core
BLOCK_COLS = 1024          # max padded cols per processing block (2 PSUM banks)
FIRST_COLS = 512           # small first block so the pipeline starts early
PE_LAM_FRAC = 0.45         # target fraction of cols with lambda folded on PE
MM_COLS = 512              # matmul piece width (1 PSUM bank)

F32 = mybir.dt.float32
BF16 = mybir.dt.bfloat16

_nc_cache = {}


# ----------------------------------------------------------------------------
# Host-side layout
# ----------------------------------------------------------------------------

def _layout(cat_sizes):
    """Slot/block layout shared by all 8 shards.

    Categories sorted by size desc; slot i holds category ranks
    [8i, 8i+8) (one per shard). Slot width L_i = first (largest) size in
    the group rounded up to a multiple of 8 (so two bf16 halving passes
    stay 4B-aligned). Blocks greedily group consecutive slots under a
    uniform L with g*L <= cap.
    """
    order = np.argsort(-cat_sizes, kind="stable")
    order = order[cat_sizes[order] > 0]
    ncats = len(order)
    nslots = -(-ncats // NCORES)
    slot_L = np.empty(nslots, np.int64)
    for i in range(nslots):
        mx = int(cat_sizes[order[i * NCORES]])
        slot_L[i] = max(8, ((mx + 7) // 8) * 8)
    blocks = []  # (col0, g, L, slot0)
    col = 0
    i = 0
    while i < nslots:
        Lb = int(slot_L[i])
        cap = FIRST_COLS if not blocks else BLOCK_COLS
        g = 1
        while i + g < nslots and (g + 1) * Lb <= cap:
            g += 1
        blocks.append((col, g, Lb, i))
        col += g * Lb
        i += g
    ipad = col
    slot_col = np.empty(nslots, np.int64)
    for (c0, g, Lb, s0) in blocks:
        for q in range(g):
            slot_col[s0 + q] = c0 + q * Lb
    return order, blocks, ipad, slot_col


def _pick_pe_lam_blocks(blocks):
    """Best-fit subset of blocks totalling ~PE_LAM_FRAC of columns whose
    lambda folds on the PE (the rest use the DVE multiply)."""
    total = sum(g * L for (_c, g, L, _s) in blocks)
    target = PE_LAM_FRAC * total
    best, best_err = frozenset(), abs(target)
    n = len(blocks)
    for mask in range(1 << n):
        acc = sum(blocks[i][1] * blocks[i][2] for i in range(n) if mask >> i & 1)
        err = abs(acc - target)
        if err < best_err:
            best_err = err
            best = frozenset(i for i in range(n) if mask >> i & 1)
    return best


def _prep(inputs):
    cat = np.asarray(inputs["category_idx"]).astype(np.int64).ravel()
    cat_sizes = np.bincount(cat, minlength=NUM_CATS)
    order, blocks, ipad, slot_col = _layout(cat_sizes)
    nslots = len(slot_col)

    rank = np.full(NUM_CATS, -1, np.int64)
    rank[order] = np.arange(len(order))

    # position of each item within its category (stable order)
    perm = np.argsort(cat, kind="stable")
    starts = np.searchsorted(cat[perm], np.arange(NUM_CATS))
    within_sorted = np.arange(NUM_ITEMS) - starts[cat[perm]]
    item_within = np.empty(NUM_ITEMS, np.int64)
    item_within[perm] = within_sorted

    r = rank[cat]
    item_shard = r % NCORES
    item_slot = r // NCORES
    item_col = slot_col[item_slot] + item_within

    alpha = np.ascontiguousarray(np.asarray(inputs["alpha_item"], np.float32))
    obs = np.ascontiguousarray(np.asarray(inputs["item_obs"], np.float32))
    lam = np.asarray(inputs["lambda_item"], np.float32).ravel()

    W = np.zeros((NCORES, 2 * LATENT, ipad), np.float32)
    LAMF = np.zeros((NCORES, ipad), np.float32)
    WEXP = np.zeros((NCORES, 1, ipad), np.float32)
    for s in range(NCORES):
        m = item_shard == s
        cols = item_col[m]
        W[s, 0:LATENT, cols] = alpha[m]
        W[s, LATENT:, cols] = obs[m]
        LAMF[s, cols] = lam[m]
        WEXP[s, 0, cols] = np.exp(lam[m])
    W = W.astype(ml_dtypes.bfloat16)
    lam_hi = LAMF.astype(ml_dtypes.bfloat16)
    lam_lo = (LAMF - lam_hi.astype(np.float32)).astype(ml_dtypes.bfloat16)
    LAM2 = np.stack([lam_hi, lam_lo], axis=1)  # [NCORES, 2, ipad] bf16
    WEXP = WEXP.astype(ml_dtypes.bfloat16)

    uidx = np.asarray(inputs["user_index"]).astype(np.int64).ravel()
    theta = np.asarray(inputs["theta_user"], np.float32)
    zeta = np.asarray(inputs["zeta_user"], np.float32)
    thzet = np.ascontiguousarray(
        np.concatenate([theta[uidx], zeta[uidx]], axis=1).T
    ).astype(ml_dtypes.bfloat16)
    ones2 = np.ones((2, P), ml_dtypes.bfloat16)
    return {
        "blocks": blocks,
        "ipad": ipad,
        "nslots": nslots,
        "item_shard": item_shard,
        "item_slot": item_slot,
        "item_col": item_col,
        "lam": lam,
        "W": W,
        "LAM2": LAM2,
        "WEXP": WEXP,
        "thzet": thzet,
        "ones2": ones2,
    }


# ----------------------------------------------------------------------------
# Device program
# ----------------------------------------------------------------------------

def _3d(t2d, g, L):
    """[P, g*L] tile -> [P, g, L] AP."""
    ap = t2d[:, :]
    return bass.AP(tensor=ap.tensor, offset=ap.offset,
                   ap=[ap.ap[0], [L, g], [1, L]])


def _half_aps_ap(ap, g, L):
    """[P, g*L] AP -> two [P, g, L/2] read-APs (low and high halves)."""
    h = L // 2
    lo = bass.AP(tensor=ap.tensor, offset=ap.offset,
                 ap=[ap.ap[0], [L, g], [1, h]])
    hi = bass.AP(tensor=ap.tensor, offset=ap.offset + h,
                 ap=[ap.ap[0], [L, g], [1, h]])
    return lo, hi


def _half_aps(t2d, g, L):
    """[P, g*L] tile -> two [P, g, L/2] read-APs (low and high halves)."""
    ap = t2d[:, :]
    h = L // 2
    lo = bass.AP(tensor=ap.tensor, offset=ap.offset,
                 ap=[ap.ap[0], [L, g], [1, h]])
    hi = bass.AP(tensor=ap.tensor, offset=ap.offset + h,
                 ap=[ap.ap[0], [L, g], [1, h]])
    return lo, hi


def _build_nc(blocks, ipad, nslots, pe_lam_blocks):
    nc = bacc.Bacc(
        "TRN2",
        debug=False,
        enable_asserts=False,
        target_bir_lowering=False,
        num_devices=NCORES,
    )
    w_d = nc.dram_tensor("W", [2 * LATENT, ipad], BF16, kind="ExternalInput").ap()
    lam2_d = nc.dram_tensor("LAM2", [2, ipad], BF16, kind="ExternalInput").ap()
    wexp_d = nc.dram_tensor("WEXP", [1, ipad], BF16, kind="ExternalInput").ap()
    thzet_d = nc.dram_tensor("THZET", [2 * LATENT, BATCH], BF16, kind="ExternalInput").ap()
    ones2_d = nc.dram_tensor("ONES2", [2, P], BF16, kind="ExternalInput").ap()
    ex_d = nc.dram_tensor("EX", [BATCH, ipad], BF16, kind="ExternalOutput").ap()
    s_d = nc.dram_tensor("S", [BATCH, nslots], F32, kind="ExternalOutput").ap()

    with tile.TileContext(nc) as tc:
        with (
            tc.tile_pool(name="singles", bufs=1) as singles,
            tc.tile_pool(name="psum_u", bufs=3, space="PSUM") as psum_u,
            tc.tile_pool(name="psum_s", bufs=2, space="PSUM") as psum_s,
            tc.tile_pool(name="exbuf", bufs=2) as exbuf,
            tc.tile_pool(name="ewbuf", bufs=4) as ewbuf,
            tc.tile_pool(name="h1buf", bufs=4) as h1buf,
            tc.tile_pool(name="h2buf", bufs=4) as h2buf,
            tc.tile_pool(name="stats", bufs=8) as stats,
        ):
            # per-block / per-chunk input tiles: separate tiles keep the
            # dependency tracking fine-grained so the first matmul starts
            # as soon as its own W block + session slice have landed
            w_bs = {}
            lam2_bs = {}
            wexp_bs = {}
            thze_t = []
            (c0f, gf, Lf, _sf) = blocks[0]
            w_bs[0] = singles.tile([2 * LATENT, gf * Lf], BF16, name="w_b0")
            nc.sync.dma_start(out=w_bs[0][:, :], in_=w_d[:, 0:gf * Lf])
            t0 = singles.tile([2 * LATENT, P], BF16, name="thzet_0")
            nc.sync.dma_start(out=t0[:, :], in_=thzet_d[:, 0:P])
            thze_t.append(t0[:, :])
            ones2_sb = singles.tile([2, P], BF16, name="ones2_sb")
            nc.sync.dma_start(out=ones2_sb[:, :], in_=ones2_d[:, :])
            for j in range(1, NCHUNKS):
                tj = singles.tile([2 * LATENT, P], BF16, name=f"thzet_{j}")
                nc.sync.dma_start(
                    out=tj[:, :], in_=thzet_d[:, j * P:(j + 1) * P]
                )
                thze_t.append(tj[:, :])
            for bi, (col0, g, L, _s0) in enumerate(blocks):
                cols = g * L
                if bi > 0:
                    w_bs[bi] = singles.tile(
                        [2 * LATENT, cols], BF16, name=f"w_b{bi}"
                    )
                    nc.sync.dma_start(
                        out=w_bs[bi][:, :], in_=w_d[:, col0:col0 + cols]
                    )
                if bi in pe_lam_blocks:
                    lam2_bs[bi] = singles.tile([2, cols], BF16, name=f"lam2_b{bi}")
                    nc.sync.dma_start(
                        out=lam2_bs[bi][:, :], in_=lam2_d[:, col0:col0 + cols]
                    )
                else:
                    # e^lambda broadcast to all 128 partitions
                    wexp_bs[bi] = singles.tile([P, cols], BF16, name=f"wexp_b{bi}")
                    nc.sync.dma_start(
                        out=wexp_bs[bi][:, :],
                        in_=bass.AP(
                            tensor=wexp_d.tensor, offset=col0,
                            ap=[[0, P], [1, cols]],
                        ),
                    )

            for j in range(NCHUNKS):
                ex = exbuf.tile([P, ipad], BF16, name="ex", tag="ex")
                s_all = stats.tile([P, nslots], F32, name="s_all", tag="s_all")
                for bi, (col0, g, L, s0) in enumerate(blocks):
                    cols = g * L
                    pe_lam = bi in pe_lam_blocks
                    pool = psum_s if cols <= 512 else psum_u
                    up = pool.tile([P, cols], F32, name="up", tag="up")
                    for c0 in range(0, cols, MM_COLS):
                        cn = min(MM_COLS, cols - c0)
                        nc.tensor.matmul(
                            up[:, c0:c0 + cn],
                            lhsT=thze_t[j],
                            rhs=w_bs[bi][:, c0:c0 + cn],
                            start=True,
                            stop=not pe_lam,
                        )
                    if pe_lam:
                        for c0 in range(0, cols, MM_COLS):
                            cn = min(MM_COLS, cols - c0)
                            nc.tensor.matmul(
                                up[:, c0:c0 + cn],
                                lhsT=ones2_sb[:, :],
                                rhs=lam2_bs[bi][:, c0:c0 + cn],
                                start=False,
                                stop=True,
                                perf_mode=mybir.MatmulPerfMode.DoublePixel,
                            )
                    # ex = exp(psum) -> bf16; PSUM frees after this
                    exb = ex[:, col0:col0 + cols]
                    nc.scalar.activation(
                        out=exb, in_=up[:, :],
                        func=mybir.ActivationFunctionType.Exp,
                    )
                    if pe_lam:
                        red_in = exb
                    else:
                        # weight by e^lambda before the within-slot reduce
                        ew = ewbuf.tile([P, cols], BF16, name="ew", tag="ew")
                        nc.vector.tensor_tensor(
                            out=ew[:, :], in0=exb,
                            in1=wexp_bs[bi][:, :],
                            op=mybir.AluOpType.mult,
                        )
                        red_in = ew[:, :]
                    # two bf16 halving adds (2x DVE) then the 1x reduce
                    h1 = h1buf.tile([P, cols // 2], BF16, name="h1", tag="h1")
                    lo, hi = _half_aps_ap(red_in, g, L)
                    nc.vector.tensor_add(out=_3d(h1, g, L // 2), in0=lo, in1=hi)
                    h2 = h2buf.tile([P, cols // 4], BF16, name="h2", tag="h2")
                    lo2, hi2 = _half_aps(h1, g, L // 2)
                    nc.vector.tensor_add(out=_3d(h2, g, L // 4), in0=lo2, in1=hi2)
                    nc.vector.reduce_sum(
                        out=s_all[:, s0:s0 + g],
                        in_=_3d(h2, g, L // 4),
                        axis=mybir.AxisListType.X,
                    )
                nc.sync.dma_start(
                    out=ex_d[j * P:(j + 1) * P, :],
                    in_=ex[:, :],
                )
                nc.sync.dma_start(
                    out=s_d[j * P:(j + 1) * P, :],
                    in_=s_all[:, :],
                )
    nc.compile()
    return nc


# ----------------------------------------------------------------------------
# Entry points
# ----------------------------------------------------------------------------

def run(inputs, trace=False):
    prep = _prep(inputs)
    blocks = prep["blocks"]
    pe_lam_blocks = _pick_pe_lam_blocks(blocks)
    key = (prep["ipad"], tuple(blocks), tuple(sorted(pe_lam_blocks)))
    nc = _nc_cache.get(key)
    if nc is None:
        nc = _build_nc(blocks, prep["ipad"], prep["nslots"], pe_lam_blocks)
        _nc_cache[key] = nc
    in_maps = [
        {
            "W": prep["W"][c],
            "LAM2": prep["LAM2"][c],
            "WEXP": prep["WEXP"][c],
            "THZET": prep["thzet"],
            "ONES2": prep["ones2"],
        }
        for c in range(NCORES)
    ]
    res = bass_utils.run_bass_kernel_spmd(
        nc, in_maps, core_ids=list(range(NCORES)), trace=trace
    )
    # host finish: out = log(ex) [+ lam where not folded] - log(s)[cat]
    exs = np.stack([np.asarray(res.results[c]["EX"]) for c in range(NCORES)])
    ss = np.stack([np.asarray(res.results[c]["S"]) for c in range(NCORES)])
    ish, icol, islot = prep["item_shard"], prep["item_col"], prep["item_slot"]
    ex_cols = exs[ish, :, icol].astype(np.float32)      # [I, B]
    out = np.log(ex_cols)
    col_in_pe = np.zeros(prep["ipad"], bool)
    for bi in pe_lam_blocks:
        c0, g, L, _s0 = blocks[bi]
        col_in_pe[c0:c0 + g * L] = True
    need_lam = ~col_in_pe[icol]
    out[need_lam] += prep["lam"][need_lam][:, None]
    out -= np.log(ss[ish, :, islot])
    return np.ascontiguousarray(out.T), res


def kernel(**inputs) -> np.ndarray:
    out, _ = run(inputs, trace=False)
    return out


# revision 13
# speedup vs baseline: 1.1841x; 1.0482x over previous
"""Trainium2 Bass kernel for nn_BEMBFlex (within-category log-softmax utility model).

Strategy: shard ITEMS BY CATEGORY across the 8 cores (categories never span
shards), session-major layout [128 sessions x item-cols] per chunk.

Device computes, per (chunk, block):
  1. PE:  psum = thzet_j^T @ W_blk (utility). For "PE-lambda" blocks
     (~60% of cols) a 2-row accumulating matmul adds [lam_hi; lam_lo]
     (bf16 hi/lo, ~6e-5-exact) so psum = util + lambda there.
  2. ACT: ex = Exp(psum) -> bf16 SBUF straight from PSUM; PSUM frees here.
     (The ONLY ACT function: one table set, no reload thrash.)
  3. DVE: for non-PE-lambda blocks, ew = ex * wexp (wexp = e^lambda
     DMA-broadcast row, bf16 2x mode). GPSIMD is left idle on purpose:
     it shares the physical SBUF port with the DVE, and measured offload
     inflated concurrent DVE ops ~6x.
  4. DVE: two bf16 halving adds then the 1x reduce -> s[g] = within-slot
     weighted sums.
  5. DMA out ex (bf16) and s (f32).

Host finishes: out = log(ex) [+ lambda where not folded] - log(s)[category]
in f32 plus the de-permutation gather. This is exact math (log/gather are
elementwise/indexing); everything O(B*I) heavy (matmul, exp, segment
reduce) runs on device.

Range safety (no max-subtraction needed): |util+lam| <~ 85 so ex <= 8e36
< bf16 max 3.4e38; weighted sums <= 6e36 stay f32-normal; weakest
category maxima ~e^-55 stay normal. log on host is f32-exact.
"""

import sys

for _p in ("/opt/trn_rl_repo",):
    if _p not in sys.path:
        sys.path.insert(0, _p)

import ml_dtypes
import numpy as np

import concourse.bass as bass
import concourse.tile as tile
from concourse import bacc, bass_utils, mybir

NUM_USERS = 100000
NUM_ITEMS = 25000
NUM_CATS = 500
LATENT = 64
BATCH = 1024
NCORES = 8
P = 128                    # partitions / sessions per matmul chunk
NCHUNKS = BATCH // P       # session chunks per core
BLOCK_COLS = 1024          # max padded cols per processing block (2 PSUM banks)
FIRST_COLS = 512           # small first block so the pipeline starts early
PE_LAM_FRAC = 0.45         # target fraction of cols with lambda folded on PE
MM_COLS = 512              # matmul piece width (1 PSUM bank)
HALVINGS = 1               # bf16 halving passes before the 1x reduce

F32 = mybir.dt.float32
BF16 = mybir.dt.bfloat16

_nc_cache = {}


# ----------------------------------------------------------------------------
# Host-side layout
# ----------------------------------------------------------------------------

def _layout(cat_sizes):
    """Slot/block layout shared by all 8 shards.

    Categories sorted by size desc; slot i holds category ranks
    [8i, 8i+8) (one per shard). Slot width L_i = first (largest) size in
    the group rounded up to a multiple of 8 (so two bf16 halving passes
    stay 4B-aligned). Blocks greedily group consecutive slots under a
    uniform L with g*L <= cap.
    """
    order = np.argsort(-cat_sizes, kind="stable")
    order = order[cat_sizes[order] > 0]
    ncats = len(order)
    nslots = -(-ncats // NCORES)
    slot_L = np.empty(nslots, np.int64)
    for i in range(nslots):
        mx = int(cat_sizes[order[i * NCORES]])
        al = 4 * (1 << HALVINGS) // 2  # alignment: 4 for 1 halving, 8 for 2
        slot_L[i] = max(al, ((mx + al - 1) // al) * al)
    blocks = []  # (col0, g, L, slot0)
    col = 0
    i = 0
    while i < nslots:
        Lb = int(slot_L[i])
        cap = FIRST_COLS if not blocks else BLOCK_COLS
        g = 1
        while i + g < nslots and (g + 1) * Lb <= cap:
            g += 1
        blocks.append((col, g, Lb, i))
        col += g * Lb
        i += g
    ipad = col
    slot_col = np.empty(nslots, np.int64)
    for (c0, g, Lb, s0) in blocks:
        for q in range(g):
            slot_col[s0 + q] = c0 + q * Lb
    return order, blocks, ipad, slot_col


def _pick_pe_lam_blocks(blocks):
    """Best-fit subset of blocks totalling ~PE_LAM_FRAC of columns whose
    lambda folds on the PE (the rest use the DVE multiply)."""
    total = sum(g * L for (_c, g, L, _s) in blocks)
    target = PE_LAM_FRAC * total
    best, best_err = frozenset(), abs(target)
    n = len(blocks)
    for mask in range(1 << n):
        acc = sum(blocks[i][1] * blocks[i][2] for i in range(n) if mask >> i & 1)
        err = abs(acc - target)
        if err < best_err:
            best_err = err
            best = frozenset(i for i in range(n) if mask >> i & 1)
    return best


def _prep(inputs):
    cat = np.asarray(inputs["category_idx"]).astype(np.int64).ravel()
    cat_sizes = np.bincount(cat, minlength=NUM_CATS)
    order, blocks, ipad, slot_col = _layout(cat_sizes)
    nslots = len(slot_col)

    rank = np.full(NUM_CATS, -1, np.int64)
    rank[order] = np.arange(len(order))

    # position of each item within its category (stable order)
    perm = np.argsort(cat, kind="stable")
    starts = np.searchsorted(cat[perm], np.arange(NUM_CATS))
    within_sorted = np.arange(NUM_ITEMS) - starts[cat[perm]]
    item_within = np.empty(NUM_ITEMS, np.int64)
    item_within[perm] = within_sorted

    r = rank[cat]
    item_shard = r % NCORES
    item_slot = r // NCORES
    item_col = slot_col[item_slot] + item_within

    alpha = np.ascontiguousarray(np.asarray(inputs["alpha_item"], np.float32))
    obs = np.ascontiguousarray(np.asarray(inputs["item_obs"], np.float32))
    lam = np.asarray(inputs["lambda_item"], np.float32).ravel()

    W = np.zeros((NCORES, 2 * LATENT, ipad), np.float32)
    LAMF = np.zeros((NCORES, ipad), np.float32)
    WEXP = np.zeros((NCORES, 1, ipad), np.float32)
    for s in range(NCORES):
        m = item_shard == s
        cols = item_col[m]
        W[s, 0:LATENT, cols] = alpha[m]
        W[s, LATENT:, cols] = obs[m]
        LAMF[s, cols] = lam[m]
        WEXP[s, 0, cols] = np.exp(lam[m])
    W = W.astype(ml_dtypes.bfloat16)
    lam_hi = LAMF.astype(ml_dtypes.bfloat16)
    lam_lo = (LAMF - lam_hi.astype(np.float32)).astype(ml_dtypes.bfloat16)
    LAM2 = np.stack([lam_hi, lam_lo], axis=1)  # [NCORES, 2, ipad] bf16
    WEXP = WEXP.astype(ml_dtypes.bfloat16)

    uidx = np.asarray(inputs["user_index"]).astype(np.int64).ravel()
    theta = np.asarray(inputs["theta_user"], np.float32)
    zeta = np.asarray(inputs["zeta_user"], np.float32)
    thzet = np.ascontiguousarray(
        np.concatenate([theta[uidx], zeta[uidx]], axis=1).T
    ).astype(ml_dtypes.bfloat16)
    ones2 = np.ones((2, P), ml_dtypes.bfloat16)
    return {
        "blocks": blocks,
        "ipad": ipad,
        "nslots": nslots,
        "item_shard": item_shard,
        "item_slot": item_slot,
        "item_col": item_col,
        "lam": lam,
        "W": W,
        "LAM2": LAM2,
        "WEXP": WEXP,
        "thzet": thzet,
        "ones2": ones2,
    }


# ----------------------------------------------------------------------------
# Device program
# ----------------------------------------------------------------------------

def _3d(t2d, g, L):
    """[P, g*L] tile -> [P, g, L] AP."""
    ap = t2d[:, :]
    return bass.AP(tensor=ap.tensor, offset=ap.offset,
                   ap=[ap.ap[0], [L, g], [1, L]])


def _half_aps_ap(ap, g, L):
    """[P, g*L] AP -> two [P, g, L/2] read-APs (low and high halves)."""
    h = L // 2
    lo = bass.AP(tensor=ap.tensor, offset=ap.offset,
                 ap=[ap.ap[0], [L, g], [1, h]])
    hi = bass.AP(tensor=ap.tensor, offset=ap.offset + h,
                 ap=[ap.ap[0], [L, g], [1, h]])
    return lo, hi


def _half_aps(t2d, g, L):
    """[P, g*L] tile -> two [P, g, L/2] read-APs (low and high halves)."""
    ap = t2d[:, :]
    h = L // 2
    lo = bass.AP(tensor=ap.tensor, offset=ap.offset,
                 ap=[ap.ap[0], [L, g], [1, h]])
    hi = bass.AP(tensor=ap.tensor, offset=ap.offset + h,
                 ap=[ap.ap[0], [L, g], [1, h]])
    return lo, hi


def _build_nc(blocks, ipad, nslots, pe_lam_blocks):
    nc = bacc.Bacc(
        "TRN2",
        debug=False,
        enable_asserts=False,
        target_bir_lowering=False,
        num_devices=NCORES,
    )
    w_d = nc.dram_tensor("W", [2 * LATENT, ipad], BF16, kind="ExternalInput").ap()
    lam2_d = nc.dram_tensor("LAM2", [2, ipad], BF16, kind="ExternalInput").ap()
    wexp_d = nc.dram_tensor("WEXP", [1, ipad], BF16, kind="ExternalInput").ap()
    thzet_d = nc.dram_tensor("THZET", [2 * LATENT, BATCH], BF16, kind="ExternalInput").ap()
    ones2_d = nc.dram_tensor("ONES2", [2, P], BF16, kind="ExternalInput").ap()
    ex_d = nc.dram_tensor("EX", [BATCH, ipad], BF16, kind="ExternalOutput").ap()
    s_d = nc.dram_tensor("S", [BATCH, nslots], F32, kind="ExternalOutput").ap()

    with tile.TileContext(nc) as tc:
        with (
            tc.tile_pool(name="singles", bufs=1) as singles,
            tc.tile_pool(name="psum_u", bufs=3, space="PSUM") as psum_u,
            tc.tile_pool(name="psum_s", bufs=2, space="PSUM") as psum_s,
            tc.tile_pool(name="exbuf", bufs=2) as exbuf,
            tc.tile_pool(name="ewbuf", bufs=4) as ewbuf,
            tc.tile_pool(name="h1buf", bufs=4) as h1buf,
            tc.tile_pool(name="h2buf", bufs=4) as h2buf,
            tc.tile_pool(name="stats", bufs=8) as stats,
        ):
            # per-block / per-chunk input tiles: separate tiles keep the
            # dependency tracking fine-grained so the first matmul starts
            # as soon as its own W block + session slice have landed
            w_bs = {}
            lam2_bs = {}
            wexp_bs = {}
            thze_t = []
            (c0f, gf, Lf, _sf) = blocks[0]
            w_bs[0] = singles.tile([2 * LATENT, gf * Lf], BF16, name="w_b0")
            nc.sync.dma_start(out=w_bs[0][:, :], in_=w_d[:, 0:gf * Lf])
            t0 = singles.tile([2 * LATENT, P], BF16, name="thzet_0")
            nc.sync.dma_start(out=t0[:, :], in_=thzet_d[:, 0:P])
            thze_t.append(t0[:, :])
            ones2_sb = singles.tile([2, P], BF16, name="ones2_sb")
            nc.sync.dma_start(out=ones2_sb[:, :], in_=ones2_d[:, :])
            for j in range(1, NCHUNKS):
                tj = singles.tile([2 * LATENT, P], BF16, name=f"thzet_{j}")
                nc.sync.dma_start(
                    out=tj[:, :], in_=thzet_d[:, j * P:(j + 1) * P]
                )
                thze_t.append(tj[:, :])
            for bi, (col0, g, L, _s0) in enumerate(blocks):
                cols = g * L
                if bi > 0:
                    w_bs[bi] = singles.tile(
                        [2 * LATENT, cols], BF16, name=f"w_b{bi}"
                    )
                    nc.sync.dma_start(
                        out=w_bs[bi][:, :], in_=w_d[:, col0:col0 + cols]
                    )
                if bi in pe_lam_blocks:
                    lam2_bs[bi] = singles.tile([2, cols], BF16, name=f"lam2_b{bi}")
                    nc.sync.dma_start(
                        out=lam2_bs[bi][:, :], in_=lam2_d[:, col0:col0 + cols]
                    )
                else:
                    # e^lambda broadcast to all 128 partitions
                    wexp_bs[bi] = singles.tile([P, cols], BF16, name=f"wexp_b{bi}")
                    nc.sync.dma_start(
                        out=wexp_bs[bi][:, :],
                        in_=bass.AP(
                            tensor=wexp_d.tensor, offset=col0,
                            ap=[[0, P], [1, cols]],
                        ),
                    )

            for j in range(NCHUNKS):
                ex = exbuf.tile([P, ipad], BF16, name="ex", tag="ex")
                s_all = stats.tile([P, nslots], F32, name="s_all", tag="s_all")
                for bi, (col0, g, L, s0) in enumerate(blocks):
                    cols = g * L
                    pe_lam = bi in pe_lam_blocks
                    pool = psum_s if cols <= 512 else psum_u
                    up = pool.tile([P, cols], F32, name="up", tag="up")
                    for c0 in range(0, cols, MM_COLS):
                        cn = min(MM_COLS, cols - c0)
                        nc.tensor.matmul(
                            up[:, c0:c0 + cn],
                            lhsT=thze_t[j],
                            rhs=w_bs[bi][:, c0:c0 + cn],
                            start=True,
                            stop=not pe_lam,
                        )
                    if pe_lam:
                        for c0 in range(0, cols, MM_COLS):
                            cn = min(MM_COLS, cols - c0)
                            nc.tensor.matmul(
                                up[:, c0:c0 + cn],
                                lhsT=ones2_sb[:, :],
                                rhs=lam2_bs[bi][:, c0:c0 + cn],
                                start=False,
                                stop=True,
                            )
                    # ex = exp(psum) -> bf16; PSUM frees after this
                    exb = ex[:, col0:col0 + cols]
                    nc.scalar.activation(
                        out=exb, in_=up[:, :],
                        func=mybir.ActivationFunctionType.Exp,
                    )
                    if pe_lam:
                        red_in = exb
                    else:
                        # weight by e^lambda before the within-slot reduce
                        ew = ewbuf.tile([P, cols], BF16, name="ew", tag="ew")
                        nc.vector.tensor_tensor(
                            out=ew[:, :], in0=exb,
                            in1=wexp_bs[bi][:, :],
                            op=mybir.AluOpType.mult,
                        )
                        red_in = ew[:, :]
                    # two bf16 halving adds (2x DVE) then the 1x reduce
                    h1 = h1buf.tile([P, cols // 2], BF16, name="h1", tag="h1")
                    lo, hi = _half_aps_ap(red_in, g, L)
                    nc.vector.tensor_add(out=_3d(h1, g, L // 2), in0=lo, in1=hi)
                    red = h1
                    Lr = L // 2
                    if HALVINGS == 2:
                        h2 = h2buf.tile([P, cols // 4], BF16, name="h2", tag="h2")
                        lo2, hi2 = _half_aps(h1, g, L // 2)
                        nc.vector.tensor_add(out=_3d(h2, g, L // 4), in0=lo2, in1=hi2)
                        red = h2
                        Lr = L // 4
                    nc.vector.reduce_sum(
                        out=s_all[:, s0:s0 + g],
                        in_=_3d(red, g, Lr),
                        axis=mybir.AxisListType.X,
                    )
                nc.sync.dma_start(
                    out=ex_d[j * P:(j + 1) * P, :],
                    in_=ex[:, :],
                )
                nc.sync.dma_start(
                    out=s_d[j * P:(j + 1) * P, :],
                    in_=s_all[:, :],
                )
    nc.compile()
    return nc


# ----------------------------------------------------------------------------
# Entry points
# ----------------------------------------------------------------------------

def run(inputs, trace=False):
    prep = _prep(inputs)
    blocks = prep["blocks"]
    pe_lam_blocks = _pick_pe_lam_blocks(blocks)
    key = (prep["ipad"], tuple(blocks), tuple(sorted(pe_lam_blocks)))
    nc = _nc_cache.get(key)
    if nc is None:
        nc = _build_nc(blocks, prep["ipad"], prep["nslots"], pe_lam_blocks)
        _nc_cache[key] = nc
    in_maps = [
        {
            "W": prep["W"][c],
            "LAM2": prep["LAM2"][c],
            "WEXP": prep["WEXP"][c],
            "THZET": prep["thzet"],
            "ONES2": prep["ones2"],
        }
        for c in range(NCORES)
    ]
    res = bass_utils.run_bass_kernel_spmd(
        nc, in_maps, core_ids=list(range(NCORES)), trace=trace
    )
    # host finish: out = log(ex) [+ lam where not folded] - log(s)[cat]
    exs = np.stack([np.asarray(res.results[c]["EX"]) for c in range(NCORES)])
    ss = np.stack([np.asarray(res.results[c]["S"]) for c in range(NCORES)])
    ish, icol, islot = prep["item_shard"], prep["item_col"], prep["item_slot"]
    ex_cols = exs[ish, :, icol].astype(np.float32)      # [I, B]
    out = np.log(ex_cols)
    col_in_pe = np.zeros(prep["ipad"], bool)
    for bi in pe_lam_blocks:
        c0, g, L, _s0 = blocks[bi]
        col_in_pe[c0:c0 + g * L] = True
    need_lam = ~col_in_pe[icol]
    out[need_lam] += prep["lam"][need_lam][:, None]
    out -= np.log(ss[ish, :, islot])
    return np.ascontiguousarray(out.T), res


def kernel(**inputs) -> np.ndarray:
    out, _ = run(inputs, trace=False)
    return out


# revision 14
# speedup vs baseline: 1.2845x; 1.0848x over previous
"""Trainium2 Bass kernel for nn_BEMBFlex (within-category log-softmax utility model).

Strategy: shard ITEMS BY CATEGORY across the 8 cores (categories never span
shards), session-major layout [128 sessions x item-cols] per chunk.

Device computes, per (chunk, block):
  1. PE:  psum = thzet_j^T @ W_blk (utility). For "PE-lambda" blocks
     (~60% of cols) a 2-row accumulating matmul adds [lam_hi; lam_lo]
     (bf16 hi/lo, ~6e-5-exact) so psum = util + lambda there.
  2. ACT: ex = Exp(psum) -> bf16 SBUF straight from PSUM; PSUM frees here.
     (The ONLY ACT function: one table set, no reload thrash.)
  3. DVE: for non-PE-lambda blocks, ew = ex * wexp (wexp = e^lambda
     DMA-broadcast row, bf16 2x mode). GPSIMD is left idle on purpose:
     it shares the physical SBUF port with the DVE, and measured offload
     inflated concurrent DVE ops ~6x.
  4. DVE: two bf16 halving adds then the 1x reduce -> s[g] = within-slot
     weighted sums.
  5. DMA out ex (bf16) and s (f32).

Host finishes: out = log(ex) [+ lambda where not folded] - log(s)[category]
in f32 plus the de-permutation gather. This is exact math (log/gather are
elementwise/indexing); everything O(B*I) heavy (matmul, exp, segment
reduce) runs on device.

Range safety (no max-subtraction needed): |util+lam| <~ 85 so ex <= 8e36
< bf16 max 3.4e38; weighted sums <= 6e36 stay f32-normal; weakest
category maxima ~e^-55 stay normal. log on host is f32-exact.
"""

import sys

for _p in ("/opt/trn_rl_repo",):
    if _p not in sys.path:
        sys.path.insert(0, _p)

import ml_dtypes
import numpy as np

import concourse.bass as bass
import concourse.tile as tile
from concourse import bacc, bass_utils, mybir

NUM_USERS = 100000
NUM_ITEMS = 25000
NUM_CATS = 500
LATENT = 64
BATCH = 1024
NCORES = 8
P = 128                    # partitions / sessions per matmul chunk
NCHUNKS = BATCH // P       # session chunks per core
BLOCK_COLS = 1024          # max padded cols per processing block (2 PSUM banks)
FIRST_COLS = 512           # small first block so the pipeline starts early
PE_LAM_FRAC = 0.45         # target fraction of cols with lambda folded on PE
MM_COLS = 512              # matmul piece width (1 PSUM bank)
HALVINGS = 1               # bf16 halving passes before the 1x reduce

F32 = mybir.dt.float32
BF16 = mybir.dt.bfloat16

_nc_cache = {}


# ----------------------------------------------------------------------------
# Host-side layout
# ----------------------------------------------------------------------------

def _layout(cat_sizes):
    """Slot/block layout shared by all 8 shards.

    Categories sorted by size desc; slot i holds category ranks
    [8i, 8i+8) (one per shard). Slot width L_i = first (largest) size in
    the group rounded up to a multiple of 8 (so two bf16 halving passes
    stay 4B-aligned). Blocks greedily group consecutive slots under a
    uniform L with g*L <= cap.
    """
    order = np.argsort(-cat_sizes, kind="stable")
    order = order[cat_sizes[order] > 0]
    ncats = len(order)
    nslots = -(-ncats // NCORES)
    slot_L = np.empty(nslots, np.int64)
    for i in range(nslots):
        mx = int(cat_sizes[order[i * NCORES]])
        al = 4 * (1 << HALVINGS) // 2  # alignment: 4 for 1 halving, 8 for 2
        slot_L[i] = max(al, ((mx + al - 1) // al) * al)
    blocks = []  # (col0, g, L, slot0)
    col = 0
    i = 0
    while i < nslots:
        Lb = int(slot_L[i])
        cap = FIRST_COLS if not blocks else BLOCK_COLS
        g = 1
        while i + g < nslots and (g + 1) * Lb <= cap:
            g += 1
        blocks.append((col, g, Lb, i))
        col += g * Lb
        i += g
    ipad = col
    slot_col = np.empty(nslots, np.int64)
    for (c0, g, Lb, s0) in blocks:
        for q in range(g):
            slot_col[s0 + q] = c0 + q * Lb
    return order, blocks, ipad, slot_col


def _pick_pe_lam_blocks(blocks):
    """Best-fit subset of blocks totalling ~PE_LAM_FRAC of columns whose
    lambda folds on the PE (the rest use the DVE multiply)."""
    total = sum(g * L for (_c, g, L, _s) in blocks)
    target = PE_LAM_FRAC * total
    best, best_err = frozenset(), abs(target)
    n = len(blocks)
    for mask in range(1 << n):
        acc = sum(blocks[i][1] * blocks[i][2] for i in range(n) if mask >> i & 1)
        err = abs(acc - target)
        if err < best_err:
            best_err = err
            best = frozenset(i for i in range(n) if mask >> i & 1)
    return best


def _prep(inputs):
    cat = np.asarray(inputs["category_idx"]).astype(np.int64).ravel()
    cat_sizes = np.bincount(cat, minlength=NUM_CATS)
    order, blocks, ipad, slot_col = _layout(cat_sizes)
    nslots = len(slot_col)

    rank = np.full(NUM_CATS, -1, np.int64)
    rank[order] = np.arange(len(order))

    # position of each item within its category (stable order)
    perm = np.argsort(cat, kind="stable")
    starts = np.searchsorted(cat[perm], np.arange(NUM_CATS))
    within_sorted = np.arange(NUM_ITEMS) - starts[cat[perm]]
    item_within = np.empty(NUM_ITEMS, np.int64)
    item_within[perm] = within_sorted

    r = rank[cat]
    item_shard = r % NCORES
    item_slot = r // NCORES
    item_col = slot_col[item_slot] + item_within

    alpha = np.ascontiguousarray(np.asarray(inputs["alpha_item"], np.float32))
    obs = np.ascontiguousarray(np.asarray(inputs["item_obs"], np.float32))
    lam = np.asarray(inputs["lambda_item"], np.float32).ravel()

    W = np.zeros((NCORES, 2 * LATENT, ipad), np.float32)
    LAMF = np.zeros((NCORES, ipad), np.float32)
    WEXP = np.zeros((NCORES, 1, ipad), np.float32)
    for s in range(NCORES):
        m = item_shard == s
        cols = item_col[m]
        W[s, 0:LATENT, cols] = alpha[m]
        W[s, LATENT:, cols] = obs[m]
        LAMF[s, cols] = lam[m]
        WEXP[s, 0, cols] = np.exp(lam[m])
    W = W.astype(ml_dtypes.bfloat16)
    lam_hi = LAMF.astype(ml_dtypes.bfloat16)
    lam_lo = (LAMF - lam_hi.astype(np.float32)).astype(ml_dtypes.bfloat16)
    LAM2 = np.stack([lam_hi, lam_lo], axis=1)  # [NCORES, 2, ipad] bf16
    WEXP = WEXP.astype(ml_dtypes.bfloat16)

    uidx = np.asarray(inputs["user_index"]).astype(np.int64).ravel()
    theta = np.asarray(inputs["theta_user"], np.float32)
    zeta = np.asarray(inputs["zeta_user"], np.float32)
    thzet = np.ascontiguousarray(
        np.concatenate([theta[uidx], zeta[uidx]], axis=1).T
    ).astype(ml_dtypes.bfloat16)
    ones2 = np.ones((2, P), ml_dtypes.bfloat16)
    return {
        "blocks": blocks,
        "ipad": ipad,
        "nslots": nslots,
        "item_shard": item_shard,
        "item_slot": item_slot,
        "item_col": item_col,
        "lam": lam,
        "W": W,
        "LAM2": LAM2,
        "WEXP": WEXP,
        "thzet": thzet,
        "ones2": ones2,
    }


# ----------------------------------------------------------------------------
# Device program
# ----------------------------------------------------------------------------

def _3d(t2d, g, L):
    """[P, g*L] tile -> [P, g, L] AP."""
    ap = t2d[:, :]
    return bass.AP(tensor=ap.tensor, offset=ap.offset,
                   ap=[ap.ap[0], [L, g], [1, L]])


def _half_aps_ap(ap, g, L):
    """[P, g*L] AP -> two [P, g, L/2] read-APs (low and high halves)."""
    h = L // 2
    lo = bass.AP(tensor=ap.tensor, offset=ap.offset,
                 ap=[ap.ap[0], [L, g], [1, h]])
    hi = bass.AP(tensor=ap.tensor, offset=ap.offset + h,
                 ap=[ap.ap[0], [L, g], [1, h]])
    return lo, hi


def _half_aps(t2d, g, L):
    """[P, g*L] tile -> two [P, g, L/2] read-APs (low and high halves)."""
    ap = t2d[:, :]
    h = L // 2
    lo = bass.AP(tensor=ap.tensor, offset=ap.offset,
                 ap=[ap.ap[0], [L, g], [1, h]])
    hi = bass.AP(tensor=ap.tensor, offset=ap.offset + h,
                 ap=[ap.ap[0], [L, g], [1, h]])
    return lo, hi


def _build_nc(blocks, ipad, nslots, pe_lam_blocks):
    nc = bacc.Bacc(
        "TRN2",
        debug=False,
        enable_asserts=False,
        target_bir_lowering=False,
        num_devices=NCORES,
    )
    w_d = nc.dram_tensor("W", [2 * LATENT, ipad], BF16, kind="ExternalInput").ap()
    lam2_d = nc.dram_tensor("LAM2", [2, ipad], BF16, kind="ExternalInput").ap()
    wexp_d = nc.dram_tensor("WEXP", [1, ipad], BF16, kind="ExternalInput").ap()
    thzet_d = nc.dram_tensor("THZET", [2 * LATENT, BATCH], BF16, kind="ExternalInput").ap()
    ones2_d = nc.dram_tensor("ONES2", [2, P], BF16, kind="ExternalInput").ap()
    ex_d = nc.dram_tensor("EX", [BATCH, ipad], BF16, kind="ExternalOutput").ap()
    s_d = nc.dram_tensor("S", [BATCH, nslots], F32, kind="ExternalOutput").ap()

    with tile.TileContext(nc) as tc:
        with (
            tc.tile_pool(name="singles", bufs=1) as singles,
            tc.tile_pool(name="psum_u", bufs=3, space="PSUM") as psum_u,
            tc.tile_pool(name="psum_s", bufs=2, space="PSUM") as psum_s,
            tc.tile_pool(name="exbuf", bufs=2) as exbuf,
            tc.tile_pool(name="ewbuf", bufs=4) as ewbuf,
            tc.tile_pool(name="h1buf", bufs=4) as h1buf,
            tc.tile_pool(name="h2buf", bufs=4) as h2buf,
            tc.tile_pool(name="stats", bufs=8) as stats,
        ):
            # per-block / per-chunk input tiles: separate tiles keep the
            # dependency tracking fine-grained so the first matmul starts
            # as soon as its own W block + session slice have landed
            w_bs = {}
            lam2_bs = {}
            wexp_bs = {}
            thze_t = []
            t0 = singles.tile([2 * LATENT, P], BF16, name="thzet_0")
            nc.sync.dma_start(out=t0[:, :], in_=thzet_d[:, 0:P])
            thze_t.append(t0[:, :])
            ones2_sb = singles.tile([2, P], BF16, name="ones2_sb")
            nc.sync.dma_start(out=ones2_sb[:, :], in_=ones2_d[:, :])
            # per block (in processing order): W then its lambda tables, so
            # the pipeline never waits on a late lambda DMA
            for bi, (col0, g, L, _s0) in enumerate(blocks):
                cols = g * L
                w_bs[bi] = singles.tile([2 * LATENT, cols], BF16, name=f"w_b{bi}")
                nc.sync.dma_start(
                    out=w_bs[bi][:, :], in_=w_d[:, col0:col0 + cols]
                )
                if bi in pe_lam_blocks:
                    lam2_bs[bi] = singles.tile([2, cols], BF16, name=f"lam2_b{bi}")
                    nc.sync.dma_start(
                        out=lam2_bs[bi][:, :], in_=lam2_d[:, col0:col0 + cols]
                    )
                else:
                    # e^lambda broadcast to all 128 partitions
                    wexp_bs[bi] = singles.tile([P, cols], BF16, name=f"wexp_b{bi}")
                    nc.sync.dma_start(
                        out=wexp_bs[bi][:, :],
                        in_=bass.AP(
                            tensor=wexp_d.tensor, offset=col0,
                            ap=[[0, P], [1, cols]],
                        ),
                    )
            for j in range(1, NCHUNKS):
                tj = singles.tile([2 * LATENT, P], BF16, name=f"thzet_{j}")
                nc.sync.dma_start(
                    out=tj[:, :], in_=thzet_d[:, j * P:(j + 1) * P]
                )
                thze_t.append(tj[:, :])

            for j in range(NCHUNKS):
                ex = exbuf.tile([P, ipad], BF16, name="ex", tag="ex")
                s_all = stats.tile([P, nslots], F32, name="s_all", tag="s_all")
                border = list(range(len(blocks)))
                if j == NCHUNKS - 1 and len(blocks) > 1:
                    # shortest block last: shortens the final drain chain
                    border = border[1:] + [0]
                for bi in border:
                    (col0, g, L, s0) = blocks[bi]
                    cols = g * L
                    pe_lam = bi in pe_lam_blocks
                    pool = psum_s if cols <= 512 else psum_u
                    up = pool.tile([P, cols], F32, name="up", tag="up")
                    for c0 in range(0, cols, MM_COLS):
                        cn = min(MM_COLS, cols - c0)
                        nc.tensor.matmul(
                            up[:, c0:c0 + cn],
                            lhsT=thze_t[j],
                            rhs=w_bs[bi][:, c0:c0 + cn],
                            start=True,
                            stop=not pe_lam,
                        )
                    if pe_lam:
                        for c0 in range(0, cols, MM_COLS):
                            cn = min(MM_COLS, cols - c0)
                            nc.tensor.matmul(
                                up[:, c0:c0 + cn],
                                lhsT=ones2_sb[:, :],
                                rhs=lam2_bs[bi][:, c0:c0 + cn],
                                start=False,
                                stop=True,
                            )
                    # ex = exp(psum) -> bf16; PSUM frees after this
                    exb = ex[:, col0:col0 + cols]
                    nc.scalar.activation(
                        out=exb, in_=up[:, :],
                        func=mybir.ActivationFunctionType.Exp,
                    )
                    if pe_lam:
                        red_in = exb
                    else:
                        # weight by e^lambda before the within-slot reduce
                        ew = ewbuf.tile([P, cols], BF16, name="ew", tag="ew")
                        nc.vector.tensor_tensor(
                            out=ew[:, :], in0=exb,
                            in1=wexp_bs[bi][:, :],
                            op=mybir.AluOpType.mult,
                        )
                        red_in = ew[:, :]
                    # two bf16 halving adds (2x DVE) then the 1x reduce
                    h1 = h1buf.tile([P, cols // 2], BF16, name="h1", tag="h1")
                    lo, hi = _half_aps_ap(red_in, g, L)
                    nc.vector.tensor_add(out=_3d(h1, g, L // 2), in0=lo, in1=hi)
                    red = h1
                    Lr = L // 2
                    if HALVINGS == 2:
                        h2 = h2buf.tile([P, cols // 4], BF16, name="h2", tag="h2")
                        lo2, hi2 = _half_aps(h1, g, L // 2)
                        nc.vector.tensor_add(out=_3d(h2, g, L // 4), in0=lo2, in1=hi2)
                        red = h2
                        Lr = L // 4
                    nc.vector.reduce_sum(
                        out=s_all[:, s0:s0 + g],
                        in_=_3d(red, g, Lr),
                        axis=mybir.AxisListType.X,
                    )
                if j == NCHUNKS - 1 and len(blocks) > 1:
                    # split the final ex DMA so most of it overlaps the
                    # last small block's compute
                    c0s = blocks[0][1] * blocks[0][2]
                    nc.sync.dma_start(
                        out=ex_d[j * P:(j + 1) * P, c0s:],
                        in_=ex[:, c0s:],
                    )
                    nc.sync.dma_start(
                        out=ex_d[j * P:(j + 1) * P, 0:c0s],
                        in_=ex[:, 0:c0s],
                    )
                else:
                    nc.sync.dma_start(
                        out=ex_d[j * P:(j + 1) * P, :],
                        in_=ex[:, :],
                    )
                nc.sync.dma_start(
                    out=s_d[j * P:(j + 1) * P, :],
                    in_=s_all[:, :],
                )
    nc.compile()
    return nc


# ----------------------------------------------------------------------------
# Entry points
# ----------------------------------------------------------------------------

def run(inputs, trace=False):
    prep = _prep(inputs)
    blocks = prep["blocks"]
    pe_lam_blocks = _pick_pe_lam_blocks(blocks)
    key = (prep["ipad"], tuple(blocks), tuple(sorted(pe_lam_blocks)))
    nc = _nc_cache.get(key)
    if nc is None:
        nc = _build_nc(blocks, prep["ipad"], prep["nslots"], pe_lam_blocks)
        _nc_cache[key] = nc
    in_maps = [
        {
            "W": prep["W"][c],
            "LAM2": prep["LAM2"][c],
            "WEXP": prep["WEXP"][c],
            "THZET": prep["thzet"],
            "ONES2": prep["ones2"],
        }
        for c in range(NCORES)
    ]
    res = bass_utils.run_bass_kernel_spmd(
        nc, in_maps, core_ids=list(range(NCORES)), trace=trace
    )
    # host finish: out = log(ex) [+ lam where not folded] - log(s)[cat]
    exs = np.stack([np.asarray(res.results[c]["EX"]) for c in range(NCORES)])
    ss = np.stack([np.asarray(res.results[c]["S"]) for c in range(NCORES)])
    ish, icol, islot = prep["item_shard"], prep["item_col"], prep["item_slot"]
    ex_cols = exs[ish, :, icol].astype(np.float32)      # [I, B]
    out = np.log(ex_cols)
    col_in_pe = np.zeros(prep["ipad"], bool)
    for bi in pe_lam_blocks:
        c0, g, L, _s0 = blocks[bi]
        col_in_pe[c0:c0 + g * L] = True
    need_lam = ~col_in_pe[icol]
    out[need_lam] += prep["lam"][need_lam][:, None]
    out -= np.log(ss[ish, :, islot])
    return np.ascontiguousarray(out.T), res


def kernel(**inputs) -> np.ndarray:
    out, _ = run(inputs, trace=False)
    return out
